# revision 21
# baseline (speedup 1.0000x reference)
"""Trainium2 Bass kernel for DebiasSoftConLoss, 8-way sharded with
symmetric-half computation of the softmax denominators.

Math (forward only; B=4096, V=2, D=128, N=V*B=8192, T=0.07):
  C = cat(unbind(features,1))            # [N, D], L2-normalized rows
  E[i,j] = exp((C_i.C_j - 1)/T)          # symmetric (global shift 1)
  denom_i = sum_{j!=i} E[i,j]
  L_i   = log(denom_i + 1e-9)
  loss  = mean_i [s2_i>0] * (L_i - s1_i/s2_i)
  (s1/s2 are tiny class-sum quantities, evaluated on the host in f32.)

Because E is symmetric, only the upper triangle of the 64x64 grid of
128-col chunks is computed (half the matmul + exp work).  Each computed
tile contributes to denom via row sums (DVE reduce over the exp'd tile)
and column sums (PE ones-matmul routed into a persistent [16,512] PSUM
accumulator by a one-hot lhsT whose hot column index is per-core DATA).

Uniform SPMD program: the triangle is cut at 8-chunk octet boundaries.
Full 8-chunk strips pair (row octet q) x (col octet m), q<m: the rhs
columns [8m,8m+8) are STATIC (read from a shared ct buffer); only the
lhsT row chunk varies per core (packed input data).  Diagonal blocks
and the intra-octet heads (widths 1..7) are packed per core.  Every
core runs 8 diag + 7 head + 28 full slots = 260 chunks.

The self term E_ii is removed on the host using a device-computed
exp(INVT*dii - INVT) where dii is the DVE square-sum of the same bf16
anchor values the PE saw, so the subtraction cancels exactly.
"""

import numpy as np

B = 4096
V = 2
D = 128
N = B * V
CORES = 8
TEMP = 0.07
INVT = 1.0 / TEMP
EPS = 1e-9

# ---- static slot structure (identical on every core) ----
# slots 0-7: diag (w=1); 8-14: heads w=1..7; 15-42: fulls (m,q) w=8
SLOT_W = [1] * 8 + list(range(1, 8)) + [8] * 28
NSLOT = len(SLOT_W)                      # 43
NDIAG = 8
NHEAD = 7
FULL_MQ = [(m, q) for m in range(1, 8) for q in range(m)]   # 28, m asc
assert len(FULL_MQ) == 28

# pack buffer (per-core rhs data for diag+head slots): chunk offsets
PACK_OFF = [0] * 15
o = 0
for t in range(15):
    PACK_OFF[t] = o
    o += SLOT_W[t]
PACK_CH = o                              # 36 chunks


def _slot_pieces(w):
    """column-sum pieces for a slot of width w chunks ending on an
    8-chunk boundary: (acc_col0, rhs_chunk_off, w_chunks)"""
    if w == 8:
        return [(0, 0, 4), (0, 4, 4)]
    if w <= 4:
        return [(512 - 128 * w, 0, w)]
    return [(512 - 128 * (w - 4), 0, w - 4), (0, w - 4, 4)]


CS_PIECES = []       # (slot, acc_col0, rhs_chunk_off, w_chunks)
for t in range(NDIAG, NSLOT):
    for (a, co, wc) in _slot_pieces(SLOT_W[t]):
        CS_PIECES.append((t, a, co, wc))
NCS = len(CS_PIECES)                     # 66

# ---- group packing (PSUM pool A=16 chunks, B=12 chunks, alternating) ----
# (pool_id, [slot indices]); pool A=0 (2048 f32), B=1 (1536 f32)
# fulls are slots 15+i for FULL_MQ[i]; early fulls first so compute can
# start on a small ct piece while the (bigger) pack DMA still streams.
GROUPS = [
    (1, list(range(0, 8))),      # 8 diags (only needs pack[0:8])   ( 8 ch)
    (0, [15, 16]),               # fulls (1,0) (2,0)     (16 ch)
    (1, [17]),                   # full  (2,1)           ( 8 ch)
    (0, [12, 13]),               # heads w5,w6           (11 ch)
    (1, [8, 9, 10, 11]),         # heads w1..w4          (10 ch)
    (0, [14, 18]),               # head w7 + full (3,0)  (15 ch)
]
fi = 19
pool = 1
while fi < NSLOT - 3:
    n = 1 if pool == 1 else 2
    GROUPS.append((pool, list(range(fi, min(fi + n, NSLOT - 3)))))
    fi += n
    pool ^= 1
GROUPS += [(1, [NSLOT - 3]), (0, [NSLOT - 2]), (1, [NSLOT - 1])]
assert sum(len(g[1]) for g in GROUPS) == NSLOT

_CACHE = {}


def _build_program():
    import concourse.bass as bass
    import concourse.tile as tile
    from concourse import bacc, mybir
    from concourse.bass import ds, ts

    f32 = mybir.dt.float32
    bf16 = mybir.dt.bfloat16
    AF = mybir.ActivationFunctionType
    OP = mybir.AluOpType

    nc = bacc.Bacc(None, target_bir_lowering=False)

    ct_d = nc.dram_tensor("ct", [128, N], bf16, kind="ExternalInput")
    pack_d = nc.dram_tensor("pack", [128, PACK_CH * 128], bf16,
                            kind="ExternalInput")
    lhs_d = nc.dram_tensor("lhs", [128, NHEAD * 128], bf16,
                           kind="ExternalInput")
    ohb_d = nc.dram_tensor("ohb", [128, NCS * 16], bf16, kind="ExternalInput")
    anc_d = nc.dram_tensor("anc", [128, NDIAG * 128], bf16, kind="ExternalInput")
    rs_d = nc.dram_tensor("rs", [128, NSLOT], f32, kind="ExternalOutput")
    self_d = nc.dram_tensor("selfE", [128, NDIAG], bf16, kind="ExternalOutput")
    cs_d = nc.dram_tensor("cs", [16, 512], f32, kind="ExternalOutput")

    with tile.TileContext(nc) as tc:
        with (
            tc.tile_pool(name="big", bufs=1) as big,
            tc.tile_pool(name="sm", bufs=1) as sm,
            tc.tile_pool(name="es", bufs=4) as esp,
            tc.tile_pool(name="psA", bufs=1, space="PSUM") as psA,
            tc.tile_pool(name="psB", bufs=1, space="PSUM") as psB,
            tc.tile_pool(name="pacc", bufs=1, space="PSUM") as pacc,
        ):
            # ---- input DMAs, split at group-consumption boundaries ----
            sb_ct = big.tile([128, N], bf16)
            sb_lhs = sm.tile([128, NHEAD * 128], bf16)
            sb_pack = big.tile([128, PACK_CH * 128], bf16)
            nc.sync.dma_start(out=sb_pack[:, 0:8 * 128], in_=pack_d[:, 0:8 * 128])
            nc.sync.dma_start(out=sb_ct[:, 1024:2048], in_=ct_d[:, 1024:2048])
            nc.sync.dma_start(out=sb_ct[:, 2048:3072], in_=ct_d[:, 2048:3072])
            nc.sync.dma_start(out=sb_pack[:, 8 * 128:], in_=pack_d[:, 8 * 128:])
            nc.sync.dma_start(out=sb_lhs[:, :], in_=lhs_d[:, :])
            nc.sync.dma_start(out=sb_ct[:, 3072:4096], in_=ct_d[:, 3072:4096])
            nc.sync.dma_start(out=sb_ct[:, 4096:6144], in_=ct_d[:, 4096:6144])
            nc.sync.dma_start(out=sb_ct[:, 6144:8192], in_=ct_d[:, 6144:8192])
            sb_ohb = sm.tile([128, NCS * 16], bf16)
            nc.gpsimd.dma_start(out=sb_ohb[:, :], in_=ohb_d[:, :])
            sb_anc = sm.tile([128, NDIAG * 128], bf16)
            nc.gpsimd.dma_start(out=sb_anc[:, :], in_=anc_d[:, :])

            # ---- colsum accumulator (one PSUM bank, pre-zeroed) ----
            acc = pacc.tile([16, 512], f32)
            nc.vector.memset(acc[:, :], 0.0)

            # exp bias tile: -1/T on every partition
            bneg = sm.tile([128, 1], f32)
            nc.vector.memset(bneg[:, :], -INVT)

            # ---- dii & self term (off critical path) ----
            dii = sm.tile([128, NDIAG], f32)
            for d in range(NDIAG):
                sq = esp.tile([128, 128], f32, tag="sq")
                nc.vector.scalar_tensor_tensor(
                    out=sq[:, :],
                    in0=sb_anc[:, ts(d, 128)],
                    scalar=0.0,
                    in1=sb_anc[:, ts(d, 128)],
                    op0=OP.add,
                    op1=OP.mult,
                    accum_out=dii[:, d:d + 1],
                )
            selfE = sm.tile([128, NDIAG], bf16)
            nc.scalar.activation(
                out=selfE[:, :], in_=dii[:, :], func=AF.Exp,
                bias=bneg[:, :], scale=INVT,
            )
            nc.sync.dma_start(out=self_d[:, :], in_=selfE[:, :])

            # ---- main loop over groups, software-pipelined ----
            # group i: dots -> exp; its colsums (PE) + rowsum reduces (DVE)
            # are emitted with group i+2 so the PE never waits on exp.
            rs = sm.tile([128, NSLOT], f32)
            cs_of_slot = {}
            ci = 0
            for (t, a, co, wc) in CS_PIECES:
                cs_of_slot.setdefault(t, []).append((ci, a, co, wc))
                ci += 1
            es_tiles = {}
            locs_of = {}

            def emit_group(gi):
                pool, slots = GROUPS[gi]
                cap = 2048 if pool == 0 else 1536
                pt = (psA if pool == 0 else psB).tile([128, cap], f32, tag="pt")
                locs = []
                gw = 0
                for t in slots:
                    locs.append(gw)
                    gw += SLOT_W[t] * 128
                locs_of[gi] = locs
                for t, lo in zip(slots, locs):
                    w = SLOT_W[t]
                    if t < NDIAG:
                        lhsT = sb_pack[:, ds(PACK_OFF[t] * 128, 128)]
                        rhs_src, rhs_off = sb_pack, PACK_OFF[t] * 128
                    elif t < NDIAG + NHEAD:
                        lhsT = sb_lhs[:, ts(t - NDIAG, 128)]
                        rhs_src, rhs_off = sb_pack, PACK_OFF[t] * 128
                    else:
                        m, q = FULL_MQ[t - 15]
                        lhsT = sb_pack[:, ds(PACK_OFF[q] * 128, 128)]
                        rhs_src, rhs_off = sb_ct, m * 1024
                    p0 = 0
                    while p0 < w * 128:
                        pw = min(512 - ((lo + p0) % 512), w * 128 - p0)
                        nc.tensor.matmul(
                            pt[:, lo + p0:lo + p0 + pw],
                            lhsT=lhsT,
                            rhs=rhs_src[:, rhs_off + p0:rhs_off + p0 + pw],
                            start=True,
                            stop=True,
                        )
                        p0 += pw
                es = esp.tile([128, 2048], bf16, tag="es")
                es_tiles[gi] = es
                single = len(slots) == 1
                nc.scalar.activation(
                    out=es[:, 0:gw],
                    in_=pt[:, 0:gw],
                    func=AF.Exp,
                    bias=bneg[:, :],
                    scale=INVT,
                    accum_out=rs[:, slots[0]:slots[0] + 1] if single else None,
                )

            def emit_tail(gi):
                pool, slots = GROUPS[gi]
                es = es_tiles.pop(gi)
                locs = locs_of[gi]
                for t, lo2 in zip(slots, locs):
                    for (ci, a, co, wc) in cs_of_slot.get(t, []):
                        nc.tensor.matmul(
                            acc[0:16, a:a + wc * 128],
                            lhsT=sb_ohb[:, ts(ci, 16)],
                            rhs=es[:, lo2 + co * 128:lo2 + (co + wc) * 128],
                            start=False,
                            stop=(ci == NCS - 1),
                            skip_group_check=True,
                        )
                if len(slots) > 1:
                    for t, lo2 in zip(slots, locs):
                        nc.vector.tensor_reduce(
                            out=rs[:, t:t + 1],
                            in_=es[:, lo2:lo2 + SLOT_W[t] * 128],
                            axis=mybir.AxisListType.X,
                            op=OP.add,
                        )

            NG = len(GROUPS)
            done_slots = set()
            early_dma = [False]
            for gi in range(NG):
                emit_group(gi)
                if gi >= 2:
                    emit_tail(gi - 2)
                    done_slots.update(GROUPS[gi - 2][1])
                if not early_dma[0] and all(
                    t in done_slots for t in range(32)
                ):
                    nc.sync.dma_start(out=rs_d[:, 0:32], in_=rs[:, 0:32])
                    early_dma[0] = True
            emit_tail(NG - 2)
            emit_tail(NG - 1)

            nc.sync.dma_start(out=rs_d[:, 32:], in_=rs[:, 32:])
            cs_sb = sm.tile([16, 512], f32)
            nc.vector.tensor_copy(out=cs_sb[:, :], in_=acc[:, :])
            nc.sync.dma_start(out=cs_d[:, :], in_=cs_sb[:, :])

    nc.compile()
    return nc


def _plan():
    """Per-core slot metadata: (rowchunk, colchunk0, width) per slot."""
    if "plan" in _CACHE:
        return _CACHE["plan"]
    plan = []
    for k in range(CORES):
        slots = []
        for d in range(NDIAG):                  # diag: row 8d+k
            r = 8 * d + k
            slots.append((r, r, 1))
        for w in range(1, 8):                   # head w: row 8k+(7-w)
            r = 8 * k + (7 - w)
            slots.append((r, r + 1, w))
        for (m, q) in FULL_MQ:                  # full: row 8q+k, cols [8m,..)
            slots.append((8 * q + k, 8 * m, 8))
        assert len(slots) == NSLOT
        plan.append(slots)
    _CACHE["plan"] = plan
    return plan


def _marshal(features, max_probs, labels):
    import ml_dtypes

    feats = np.ascontiguousarray(np.asarray(features, dtype=np.float32))
    C = np.ascontiguousarray(feats.transpose(1, 0, 2).reshape(N, D))
    Cb = C.astype(ml_dtypes.bfloat16)
    ct = np.ascontiguousarray(Cb.T)                      # [D, N] bf16

    plan = _plan()
    in_maps = []
    for k in range(CORES):
        slots = plan[k]
        pack = np.empty((128, PACK_CH * 128), dtype=ml_dtypes.bfloat16)
        lhs = np.empty((128, NHEAD * 128), dtype=ml_dtypes.bfloat16)
        ohb = np.zeros((128, NCS * 16), dtype=ml_dtypes.bfloat16)
        anc = np.empty((128, NDIAG * 128), dtype=ml_dtypes.bfloat16)
        for t in range(15):
            r, c0, w = slots[t]
            o = PACK_OFF[t] * 128
            pack[:, o:o + w * 128] = ct[:, c0 * 128:(c0 + w) * 128]
        for t in range(NDIAG, NDIAG + NHEAD):
            r = slots[t][0]
            lhs[:, (t - NDIAG) * 128:(t - NDIAG + 1) * 128] = \
                ct[:, r * 128:(r + 1) * 128]
        for d in range(NDIAG):
            r = slots[d][0]
            anc[:, d * 128:(d + 1) * 128] = Cb[r * 128:(r + 1) * 128, :]
        for i, (t, a, co, wc) in enumerate(CS_PIECES):
            r, c0, w = slots[t]
            gcol = (c0 + co) * 128
            s = (gcol - a) // 512
            assert 512 * s + a == gcol and 0 <= s < 16, (t, a, co, wc, gcol)
            ohb[:, i * 16 + s] = 1.0
        in_maps.append({"ct": ct, "pack": pack, "lhs": lhs, "ohb": ohb,
                        "anc": anc})
    return in_maps


def _run_raw(in_maps, **kw):
    from concourse.bass_utils import run_bass_kernel_spmd

    if "nc" not in _CACHE:
        _CACHE["nc"] = _build_program()
    return run_bass_kernel_spmd(
        _CACHE["nc"], in_maps, core_ids=list(range(CORES)), **kw
    )


def _finish(res, features, max_probs, labels):
    """Host combine: O(N*D) in f32."""
    feats = np.asarray(features, dtype=np.float32)
    C = np.ascontiguousarray(feats.transpose(1, 0, 2).reshape(N, D))
    mp = np.asarray(max_probs, dtype=np.float32).reshape(B)
    lab = np.asarray(labels).astype(np.int64).reshape(B)
    mp_full = np.tile(mp, V)
    lab_full = np.tile(lab, V)

    plan = _plan()
    denom = np.zeros(N, dtype=np.float64)
    for k in range(CORES):
        r = res.results[k]
        rs, cs = r["rs"], r["cs"]
        selfE = np.asarray(r["selfE"]).astype(np.float32)
        slots = plan[k]
        for t, (rr, c0, w) in enumerate(slots):
            denom[rr * 128:(rr + 1) * 128] += rs[:, t]
        for d in range(NDIAG):
            rr = slots[d][0]
            denom[rr * 128:(rr + 1) * 128] -= selfE[:, d]
        denom += cs.reshape(N)

    L = np.log(denom + EPS)

    S = np.zeros(10, dtype=np.float32)
    np.add.at(S, lab_full, mp_full)
    g = np.zeros((10, D), dtype=np.float32)
    np.add.at(g, lab_full, mp_full[:, None] * C)
    q = np.einsum("nd,nd->n", C, g[lab_full])
    dot_ii = np.einsum("nd,nd->n", C, C)
    Sl = S[lab_full]
    s1 = mp_full * (q - Sl - mp_full * (dot_ii - 1.0)) / TEMP
    s2 = mp_full * (Sl - mp_full)
    loss = np.where(s2 == 0, 0.0, L - s1 / np.where(s2 == 0, 1.0, s2))
    return np.float32(loss.mean())


def kernel(features, max_probs, labels):
    in_maps = _marshal(features, max_probs, labels)
    res = _run_raw(in_maps)
    return _finish(res, features, max_probs, labels)
